# revision 2
# baseline (speedup 1.0000x reference)
"""Trainium2 Bass kernel v2: embedding -> Linear -> tanh-RNN -> Linear -> sigmoid.

Structure vs v1: the per-core batch of 16 rows is split into C independent
recurrence chains (default 6,5,5). Each chain-step is one PSUM accumulation
group (identity-seeded with p_t, then 64 U-matmuls) closed by a single ACT
tanh. Chains are emitted round-robin so each chain's matmul->tanh->matmul
latency (~680ns) is hidden under the other chains' matmuls; the input
projection (embedding gather + W matmul + bias via DVE) dribbles into the
leftover PE/DVE slack.

Hardcoded problem shapes:
  x   [128, 512] int   (token ids < 32000)
  emb [32000, 512] f32
  W_w [1024, 512], W_b [1024]
  U_w [1024, 1024], U_b [1024]
  V_w [1, 1024],  V_b [1]
"""

import os
import sys

import numpy as np

sys.path.insert(0, "/opt/trn_rl_repo")

import concourse.bass as bass  # noqa: E402
from concourse import bacc  # noqa: E402
import concourse.mybir as mybir  # noqa: E402
import concourse.tile as tile  # noqa: E402
from concourse.bass_utils import run_bass_kernel_spmd  # noqa: E402

B, S, E, H, VOCAB = 128, 512, 512, 1024, 32000
NCORES = 8
BL = B // NCORES  # 16 batch rows per core
NTOK = BL * S  # 8192 tokens per core, flat order i = s*BL + b
CHUNK = int(os.environ.get("V2_CHUNK", "128"))  # tokens per gather chunk
P = 128
ET, HT, KT = E // P, H // P, H // P  # 4, 8, 8
SPC = CHUNK // BL  # steps covered per chunk

F16 = mybir.dt.float16
F32 = mybir.dt.float32
I16 = mybir.dt.int16
AF = mybir.ActivationFunctionType

CHAINS = tuple(int(c) for c in os.environ.get("V2_CHAINS", "6,5,5").split(","))
assert sum(CHAINS) == BL
NC = len(CHAINS)
CH_OFF = [sum(CHAINS[:i]) for i in range(NC)]
# jt-split per chain: tanh'd in G groups so next step's kt-lo matmuls can
# start as soon as the first group's tanh lands
JSPLIT = tuple(int(g) for g in os.environ.get("V2_JSPLIT", "8").split(","))
assert sum(JSPLIT) == HT
JOFF = [sum(JSPLIT[:i]) for i in range(len(JSPLIT))]

STEPS = int(os.environ.get("V2_STEPS", S))
GATHER_AHEAD = int(os.environ.get("V2_AHEAD", "2"))
# how many proj matmuls to insert after each chain slot (<=0: all at step end)
COPY_ACT = os.environ.get("V2_COPY_ACT", "0") == "1"  # proj copies on ACT too

_cache = {}


def _build():
    nc = bacc.Bacc(None)
    emb_d = nc.declare_dram_parameter("embt", [VOCAB, E], F16, isOutput=False)
    idx_d = nc.declare_dram_parameter("idx", [P, S], I16, isOutput=False)
    wt_d = nc.declare_dram_parameter("wt", [P, ET, H], F16, isOutput=False)
    ut_d = nc.declare_dram_parameter("ut", [P, KT, H], F16, isOutput=False)
    bias_d = nc.declare_dram_parameter("bias", [P, HT], F32, isOutput=False)
    vt_d = nc.declare_dram_parameter("vt", [P, HT], F16, isOutput=False)
    vb_d = nc.declare_dram_parameter("vb", [1, 1], F32, isOutput=False)
    ident_d = nc.declare_dram_parameter("ident", [P, P], F16, isOutput=False)
    out_d = nc.declare_dram_parameter("out", [1, BL], F32, isOutput=True)

    with tile.TileContext(nc) as tc:
        with (
            tc.tile_pool(name="const", bufs=1) as constp,
            tc.tile_pool(name="pre", bufs=1) as prep,
            tc.tile_pool(name="xe", bufs=int(os.environ.get("V2_XEBUFS", "3"))) as xep,
            tc.tile_pool(name="h", bufs=int(os.environ.get("V2_HBUFS", "2"))) as hp,
            tc.tile_pool(name="misc", bufs=1) as miscp,
        ):
            idx_sb = constp.tile([P, S], I16, tag="idx")
            nc.sync.dma_start(out=idx_sb[:], in_=idx_d[:])
            wt_sb = constp.tile([P, ET, H], F16, tag="wt")
            nc.sync.dma_start(out=wt_sb[:], in_=wt_d[:])
            ut_sb = constp.tile([P, KT, H], F16, tag="ut")
            nc.sync.dma_start(out=ut_sb[:], in_=ut_d[:])
            bias_sb = constp.tile([P, HT], F32, tag="bias")
            nc.sync.dma_start(out=bias_sb[:], in_=bias_d[:])
            vt_sb = constp.tile([P, HT], F16, tag="vt")
            nc.sync.dma_start(out=vt_sb[:], in_=vt_d[:])
            vb_sb = constp.tile([1, 1], F32, tag="vb")
            nc.sync.dma_start(out=vb_sb[:], in_=vb_d[:])
            ident_sb = constp.tile([P, P], F16, tag="ident")
            nc.sync.dma_start(out=ident_sb[:], in_=ident_d[:])

            # preT[p, ht, s*BL + b] = (xe @ W.T + W_b + U_b)[b, s, ht*128 + p]
            preT = prep.tile([P, HT, NTOK], F16, tag="preT")

            with (
                tc.tile_pool(
                    name="recps",
                    bufs=int(os.environ.get("V2_RECBUFS", "6")),
                    space=bass.MemorySpace.PSUM,
                ) as recps,
                tc.tile_pool(
                    name="projps", bufs=2, space=bass.MemorySpace.PSUM
                ) as projps,
            ):
                h_cur = [None] * NC

                def emit_gather(c):
                    xet = xep.tile([P, ET, CHUNK], F16, tag="xet")
                    nc.gpsimd.dma_gather(
                        out_ap=xet[:],
                        in_ap=emb_d[:],
                        idxs_ap=idx_sb[:, c * SPC : (c + 1) * SPC],
                        num_idxs=CHUNK,
                        num_idxs_reg=CHUNK,
                        elem_size=E,
                        transpose=True,
                    )
                    return xet

                def emit_proj_mm(xet, c, ht):
                    ps = projps.tile([P, CHUNK], F32, tag="pps")
                    for et in range(ET):
                        nc.tensor.matmul(
                            ps[:],
                            wt_sb[:, et, ht * P : (ht + 1) * P],
                            xet[:, et, 0:CHUNK],
                            start=(et == 0),
                            stop=(et == ET - 1),
                        )
                    return ps

                def emit_proj_one_mm(xet, ps, ht, et):
                    if et == 0:
                        ps = projps.tile([P, CHUNK], F32, tag="pps")
                    nc.tensor.matmul(
                        ps[:],
                        wt_sb[:, et, ht * P : (ht + 1) * P],
                        xet[:, et, 0:CHUNK],
                        start=(et == 0),
                        stop=(et == ET - 1),
                    )
                    return ps

                def emit_proj_copy(ps, c, ht):
                    toff = c * CHUNK
                    eng = nc.scalar if (COPY_ACT and ht % 2 == 0) else nc.vector
                    if eng is nc.vector:
                        nc.vector.tensor_tensor(
                            out=preT[:, ht, toff : toff + CHUNK],
                            in0=ps[:],
                            in1=bias_sb[:, ht : ht + 1].to_broadcast([P, CHUNK]),
                            op=mybir.AluOpType.add,
                        )
                    else:
                        nc.scalar.activation(
                            preT[:, ht, toff : toff + CHUNK],
                            ps[:],
                            AF.Identity,
                            bias=bias_sb[:, ht : ht + 1],
                        )

                def emit_chain_step(x, t):
                    b0, bl = CH_OFF[x], CHAINS[x]
                    if t == 0:
                        h_new = hp.tile([P, KT, bl], F16, tag=f"h{x}")
                        nc.scalar.activation(
                            h_new[:, :, :],
                            preT[:, :, b0 : b0 + bl],
                            AF.Tanh,
                        )
                        h_cur[x] = h_new
                        return
                    h_prev = h_cur[x]
                    h_new = hp.tile([P, KT, bl], F16, tag=f"h{x}")
                    toff = t * BL + b0
                    # one PSUM group + tanh per jt-group; matmuls kt-ascending so
                    # they start as soon as the previous step's first tanh lands
                    for gi, gsz in enumerate(JSPLIT):
                        j0 = JOFF[gi]
                        ps = recps.tile([P, max(JSPLIT), bl], F32, tag="recps")
                        nc.tensor.matmul(
                            ps[:, 0:gsz, :],
                            ident_sb[:],
                            preT[:, j0 : j0 + gsz, toff : toff + bl],
                            start=True,
                            stop=False,
                            skip_group_check=True,
                        )
                        n_mm = 0
                        for kt in range(KT):
                            for j in range(gsz):
                                n_mm += 1
                                nc.tensor.matmul(
                                    ps[:, j, :],
                                    ut_sb[:, kt, (j0 + j) * P : (j0 + j + 1) * P],
                                    h_prev[:, kt, :],
                                    start=False,
                                    stop=(n_mm == gsz * KT),
                                    skip_group_check=True,
                                )
                        nc.scalar.activation(
                            h_new[:, j0 : j0 + gsz, :], ps[:, 0:gsz, :], AF.Tanh
                        )
                    h_cur[x] = h_new

                # ---------------- pipelined emission ----------------
                # proj work for chunk c is emitted during steps of window c-1;
                # gather for chunk c+AHEAD-1 kicked at window start.
                NCHUNKS = NTOK // CHUNK

                # prologue: gather + project chunk 0 fully, gather chunk 1..AHEAD
                xet0 = emit_gather(0)
                for ht in range(HT):
                    ps = emit_proj_mm(xet0, 0, ht)
                    emit_proj_copy(ps, 0, ht)
                pending = []  # list of (xet, chunk, next_ht)
                for c in range(1, min(GATHER_AHEAD + 1, NCHUNKS)):
                    pending.append([emit_gather(c), c, 0])

                cur_ps = [None]

                def dribble_proj():
                    # one matmul per call; close+copy after ET of them
                    if not pending:
                        return
                    ent = pending[0]
                    xet, c, mmi = ent
                    ht, et = mmi // ET, mmi % ET
                    cur_ps[0] = emit_proj_one_mm(xet, cur_ps[0], ht, et)
                    if et == ET - 1:
                        emit_proj_copy(cur_ps[0], c, ht)
                    ent[2] += 1
                    if ent[2] == HT * ET:
                        pending.pop(0)

                # proj matmuls owed per step so gather chunks drain on schedule
                rate = (HT * ET) / SPC
                per_slot = max(1, int(rate / NC) + 1)
                owed = 0.0
                for t in range(STEPS):
                    if t % SPC == 0 and t > 0:
                        nxt = t // SPC + GATHER_AHEAD
                        if nxt < NCHUNKS:
                            pending.append([emit_gather(nxt), nxt, 0])
                    owed += rate
                    for x in range(NC):
                        emit_chain_step(x, t)
                        # insert proj matmuls between chain blocks so the
                        # tanh stream is spread across the period
                        n = 0
                        while t > 0 and owed >= 1.0 and n < per_slot:
                            dribble_proj()
                            owed -= 1.0
                            n += 1
                # drain any remaining proj work (shouldn't happen)
                while pending:
                    dribble_proj()

                # ---------------- output head ----------------
                pv = projps.tile([1, BL], F32, tag="pps")
                n_mm = 0
                for x in range(NC):
                    b0, bl = CH_OFF[x], CHAINS[x]
                    for kt in range(KT):
                        n_mm += 1
                        nc.tensor.matmul(
                            pv[0:1, b0 : b0 + bl],
                            vt_sb[:, kt : kt + 1],
                            h_cur[x][:, kt, :],
                            start=(n_mm == 1),
                            stop=(n_mm == NC * KT),
                            skip_group_check=True,
                        )
                # sigmoid(z+vb) == 0.5*tanh((z+vb)/2)+0.5; vb pre-halved, affine on host
                out_sb = miscp.tile([1, BL], F32, tag="out")
                nc.scalar.activation(out_sb[:], pv[:], AF.Tanh, bias=vb_sb[:], scale=0.5)
                nc.sync.dma_start(out=out_d[:], in_=out_sb[:])

    _strip_same_engine_waits(nc)
    nc.finalize()
    return nc


_ENGSEM = {"Activation": "Activation_", "PE": "PE_", "DVE": "DVE_", "Pool": "Pool_", "SP": "SP_"}


def _strip_same_engine_waits(nc):
    """Drop semaphore waits an instruction holds on its own engine's
    completion counter. Engines execute in order, so these waits are
    redundant for correctness; removing them lets the one remaining
    cross-engine wait attach to the instruction (instead of spilling to a
    standalone EventSemaphore that blocks the sequencer from pre-decoding),
    which cuts ~80ns of dispatch latency out of the per-step critical cycle."""
    if os.environ.get("V2_STRIP", "1") != "1":
        return
    for f in nc.m.functions:
        for b in f.blocks:
            for ins in b.instructions:
                si = ins.sync_info
                if si is None or not si.on_wait:
                    continue
                pref = _ENGSEM.get(str(ins.engine).split(".")[-1])
                if pref is None:
                    continue
                keep = [w for w in si.on_wait if not w.ant_name.startswith(pref)]
                if len(keep) != len(si.on_wait):
                    si.on_wait = keep


def kernel(x, emb, W_w, W_b, U_w, U_b, V_w, V_b):
    x = np.asarray(x)
    emb = np.asarray(emb, dtype=np.float32)
    W_w = np.asarray(W_w, dtype=np.float32)
    W_b = np.asarray(W_b, dtype=np.float32)
    U_w = np.asarray(U_w, dtype=np.float32)
    U_b = np.asarray(U_b, dtype=np.float32)
    V_w = np.asarray(V_w, dtype=np.float32)
    V_b = np.asarray(V_b, dtype=np.float32)

    if "nc" not in _cache:
        _cache["nc"] = _build()
    nc = _cache["nc"]

    bf = np.float16
    embt = np.ascontiguousarray(emb.astype(bf))
    # wt[p, et, h] = W_w.T[et*128+p, h]
    wt = np.ascontiguousarray(W_w.T.reshape(ET, P, H).transpose(1, 0, 2).astype(bf))
    # ut[p, kt, j] = U_w.T[kt*128+p, j]
    ut = np.ascontiguousarray(U_w.T.reshape(KT, P, H).transpose(1, 0, 2).astype(bf))
    bias = np.ascontiguousarray((W_b + U_b).reshape(HT, P).T.astype(np.float32))
    vt = np.ascontiguousarray(V_w[0].reshape(HT, P).T.astype(bf))
    vb = (V_b / 2.0).reshape(1, 1).astype(np.float32)
    ident = np.eye(P, dtype=np.float32).astype(bf)

    in_maps = []
    for c in range(NCORES):
        xl = np.ascontiguousarray(
            np.tile(x[c * BL : (c + 1) * BL, :].astype(np.int16), (P // BL, 1))
        )
        in_maps.append(
            {
                "embt": embt,
                "idx": xl,
                "wt": wt,
                "ut": ut,
                "bias": bias,
                "vt": vt,
                "vb": vb,
                "ident": ident,
            }
        )

    _cache["last_in_maps"] = in_maps
    res = run_bass_kernel_spmd(nc, in_maps, list(range(NCORES)))
    _cache["last_exec_time_ns"] = res.exec_time_ns
    _cache["last_results"] = res

    out = np.empty((B, 1), dtype=np.float32)
    for c in range(NCORES):
        out[c * BL : (c + 1) * BL, 0] = res.results[c]["out"][0, :]
    return 0.5 * out + 0.5


# revision 3
# speedup vs baseline: 1.0050x; 1.0050x over previous
"""Trainium2 Bass kernel v2: embedding -> Linear -> tanh-RNN -> Linear -> sigmoid.

Structure vs v1: the per-core batch of 16 rows is split into C independent
recurrence chains (default 6,5,5). Each chain-step is one PSUM accumulation
group (identity-seeded with p_t, then 64 U-matmuls) closed by a single ACT
tanh. Chains are emitted round-robin so each chain's matmul->tanh->matmul
latency (~680ns) is hidden under the other chains' matmuls; the input
projection (embedding gather + W matmul + bias via DVE) dribbles into the
leftover PE/DVE slack.

Hardcoded problem shapes:
  x   [128, 512] int   (token ids < 32000)
  emb [32000, 512] f32
  W_w [1024, 512], W_b [1024]
  U_w [1024, 1024], U_b [1024]
  V_w [1, 1024],  V_b [1]
"""

import os
import sys

import numpy as np

sys.path.insert(0, "/opt/trn_rl_repo")

import concourse.bass as bass  # noqa: E402
from concourse import bacc  # noqa: E402
import concourse.mybir as mybir  # noqa: E402
import concourse.tile as tile  # noqa: E402
from concourse.bass_utils import run_bass_kernel_spmd  # noqa: E402

B, S, E, H, VOCAB = 128, 512, 512, 1024, 32000
NCORES = 8
BL = B // NCORES  # 16 batch rows per core
NTOK = BL * S  # 8192 tokens per core, flat order i = s*BL + b
CHUNK = int(os.environ.get("V2_CHUNK", "128"))  # tokens per gather chunk
P = 128
ET, HT, KT = E // P, H // P, H // P  # 4, 8, 8
SPC = CHUNK // BL  # steps covered per chunk

F16 = mybir.dt.float16
F32 = mybir.dt.float32
I16 = mybir.dt.int16
AF = mybir.ActivationFunctionType

CHAINS = tuple(int(c) for c in os.environ.get("V2_CHAINS", "6,5,5").split(","))
assert sum(CHAINS) == BL
NC = len(CHAINS)
CH_OFF = [sum(CHAINS[:i]) for i in range(NC)]
# jt-split per chain: tanh'd in G groups so next step's kt-lo matmuls can
# start as soon as the first group's tanh lands
JSPLIT = tuple(int(g) for g in os.environ.get("V2_JSPLIT", "8").split(","))
assert sum(JSPLIT) == HT
JOFF = [sum(JSPLIT[:i]) for i in range(len(JSPLIT))]

STEPS = int(os.environ.get("V2_STEPS", S))
GATHER_AHEAD = int(os.environ.get("V2_AHEAD", "2"))
# how many proj matmuls to insert after each chain slot (<=0: all at step end)
COPY_ACT = os.environ.get("V2_COPY_ACT", "0") == "1"  # proj copies on ACT too

_cache = {}


def _build():
    nc = bacc.Bacc(None)
    emb_d = nc.declare_dram_parameter("embt", [VOCAB, E], F16, isOutput=False)
    idx_d = nc.declare_dram_parameter("idx", [P, S], I16, isOutput=False)
    wt_d = nc.declare_dram_parameter("wt", [P, ET, H], F16, isOutput=False)
    ut_d = nc.declare_dram_parameter("ut", [P, KT, H], F16, isOutput=False)
    bias_d = nc.declare_dram_parameter("bias", [P, HT], F32, isOutput=False)
    vt_d = nc.declare_dram_parameter("vt", [P, HT], F16, isOutput=False)
    vb_d = nc.declare_dram_parameter("vb", [1, 1], F32, isOutput=False)
    ident_d = nc.declare_dram_parameter("ident", [P, P], F16, isOutput=False)
    out_d = nc.declare_dram_parameter("out", [1, BL], F32, isOutput=True)

    with tile.TileContext(nc) as tc:
        with (
            tc.tile_pool(name="const", bufs=1) as constp,
            tc.tile_pool(name="pre", bufs=1) as prep,
            tc.tile_pool(name="xe", bufs=int(os.environ.get("V2_XEBUFS", "3"))) as xep,
            tc.tile_pool(name="h", bufs=int(os.environ.get("V2_HBUFS", "2"))) as hp,
            tc.tile_pool(name="misc", bufs=1) as miscp,
        ):
            # DMA order matters: everything the first projection chunk and
            # step-0 tanh needs goes first; the 2MB ut load is split per-kt
            # and issued last so it streams in under the prologue instead of
            # blocking it (DMA engine pool is serial in the cost model).
            idx_sb = constp.tile([P, S], I16, tag="idx")
            nc.sync.dma_start(out=idx_sb[:], in_=idx_d[:])
            bias_sb = constp.tile([P, HT], F32, tag="bias")
            nc.sync.dma_start(out=bias_sb[:], in_=bias_d[:])
            ident_sb = constp.tile([P, P], F16, tag="ident")
            nc.sync.dma_start(out=ident_sb[:], in_=ident_d[:])
            vt_sb = constp.tile([P, HT], F16, tag="vt")
            nc.sync.dma_start(out=vt_sb[:], in_=vt_d[:])
            vb_sb = constp.tile([1, 1], F32, tag="vb")
            nc.sync.dma_start(out=vb_sb[:], in_=vb_d[:])
            wt_sb = constp.tile([P, ET, H], F16, tag="wt")
            nc.sync.dma_start(out=wt_sb[:], in_=wt_d[:])
            ut_sb = constp.tile([P, KT, H], F16, tag="ut")
            nc.sync.dma_start(out=ut_sb[:, 0 : KT // 2, :], in_=ut_d[:, 0 : KT // 2, :])
            nc.sync.dma_start(out=ut_sb[:, KT // 2 :, :], in_=ut_d[:, KT // 2 :, :])

            # preT[p, ht, s*BL + b] = (xe @ W.T + W_b + U_b)[b, s, ht*128 + p]
            preT = prep.tile([P, HT, NTOK], F16, tag="preT")

            with (
                tc.tile_pool(
                    name="recps",
                    bufs=int(os.environ.get("V2_RECBUFS", "6")),
                    space=bass.MemorySpace.PSUM,
                ) as recps,
                tc.tile_pool(
                    name="projps", bufs=2, space=bass.MemorySpace.PSUM
                ) as projps,
            ):
                h_cur = [None] * NC

                def emit_gather(c):
                    xet = xep.tile([P, ET, CHUNK], F16, tag="xet")
                    nc.gpsimd.dma_gather(
                        out_ap=xet[:],
                        in_ap=emb_d[:],
                        idxs_ap=idx_sb[:, c * SPC : (c + 1) * SPC],
                        num_idxs=CHUNK,
                        num_idxs_reg=CHUNK,
                        elem_size=E,
                        transpose=True,
                    )
                    return xet

                def emit_proj_mm(xet, c, ht):
                    ps = projps.tile([P, CHUNK], F32, tag="pps")
                    for et in range(ET):
                        nc.tensor.matmul(
                            ps[:],
                            wt_sb[:, et, ht * P : (ht + 1) * P],
                            xet[:, et, 0:CHUNK],
                            start=(et == 0),
                            stop=(et == ET - 1),
                        )
                    return ps

                def emit_proj_one_mm(xet, ps, ht, et):
                    if et == 0:
                        ps = projps.tile([P, CHUNK], F32, tag="pps")
                    nc.tensor.matmul(
                        ps[:],
                        wt_sb[:, et, ht * P : (ht + 1) * P],
                        xet[:, et, 0:CHUNK],
                        start=(et == 0),
                        stop=(et == ET - 1),
                    )
                    return ps

                def emit_proj_copy(ps, c, ht):
                    toff = c * CHUNK
                    eng = nc.scalar if (COPY_ACT and ht % 2 == 0) else nc.vector
                    if eng is nc.vector:
                        nc.vector.tensor_tensor(
                            out=preT[:, ht, toff : toff + CHUNK],
                            in0=ps[:],
                            in1=bias_sb[:, ht : ht + 1].to_broadcast([P, CHUNK]),
                            op=mybir.AluOpType.add,
                        )
                    else:
                        nc.scalar.activation(
                            preT[:, ht, toff : toff + CHUNK],
                            ps[:],
                            AF.Identity,
                            bias=bias_sb[:, ht : ht + 1],
                        )

                def emit_chain_step(x, t):
                    b0, bl = CH_OFF[x], CHAINS[x]
                    if t == 0:
                        h_new = hp.tile([P, KT, bl], F16, tag=f"h{x}")
                        nc.scalar.activation(
                            h_new[:, :, :],
                            preT[:, :, b0 : b0 + bl],
                            AF.Tanh,
                        )
                        h_cur[x] = h_new
                        return
                    h_prev = h_cur[x]
                    h_new = hp.tile([P, KT, bl], F16, tag=f"h{x}")
                    toff = t * BL + b0
                    # one PSUM group + tanh per jt-group; matmuls kt-ascending so
                    # they start as soon as the previous step's first tanh lands
                    for gi, gsz in enumerate(JSPLIT):
                        j0 = JOFF[gi]
                        ps = recps.tile([P, max(JSPLIT), bl], F32, tag="recps")
                        nc.tensor.matmul(
                            ps[:, 0:gsz, :],
                            ident_sb[:],
                            preT[:, j0 : j0 + gsz, toff : toff + bl],
                            start=True,
                            stop=False,
                            skip_group_check=True,
                        )
                        n_mm = 0
                        for kt in range(KT):
                            for j in range(gsz):
                                n_mm += 1
                                nc.tensor.matmul(
                                    ps[:, j, :],
                                    ut_sb[:, kt, (j0 + j) * P : (j0 + j + 1) * P],
                                    h_prev[:, kt, :],
                                    start=False,
                                    stop=(n_mm == gsz * KT),
                                    skip_group_check=True,
                                )
                        nc.scalar.activation(
                            h_new[:, j0 : j0 + gsz, :], ps[:, 0:gsz, :], AF.Tanh
                        )
                    h_cur[x] = h_new

                # ---------------- pipelined emission ----------------
                # proj work for chunk c is emitted during steps of window c-1;
                # gather for chunk c+AHEAD-1 kicked at window start.
                NCHUNKS = NTOK // CHUNK

                # prologue: gather + project chunk 0 fully, gather chunk 1..AHEAD
                xet0 = emit_gather(0)
                for ht in range(HT):
                    ps = emit_proj_mm(xet0, 0, ht)
                    emit_proj_copy(ps, 0, ht)
                pending = []  # list of (xet, chunk, next_ht)
                for c in range(1, min(GATHER_AHEAD + 1, NCHUNKS)):
                    pending.append([emit_gather(c), c, 0])

                cur_ps = [None]

                def dribble_proj():
                    # one matmul per call; close+copy after ET of them
                    if not pending:
                        return
                    ent = pending[0]
                    xet, c, mmi = ent
                    ht, et = mmi // ET, mmi % ET
                    cur_ps[0] = emit_proj_one_mm(xet, cur_ps[0], ht, et)
                    if et == ET - 1:
                        emit_proj_copy(cur_ps[0], c, ht)
                    ent[2] += 1
                    if ent[2] == HT * ET:
                        pending.pop(0)

                # proj matmuls owed per step so gather chunks drain on schedule
                rate = (HT * ET) / SPC
                per_slot = max(1, int(rate / NC) + 1)
                owed = 0.0
                for t in range(STEPS):
                    if t % SPC == 0 and t > 0:
                        nxt = t // SPC + GATHER_AHEAD
                        if nxt < NCHUNKS:
                            pending.append([emit_gather(nxt), nxt, 0])
                    owed += rate
                    for x in range(NC):
                        emit_chain_step(x, t)
                        # insert proj matmuls between chain blocks so the
                        # tanh stream is spread across the period
                        n = 0
                        while t > 0 and owed >= 1.0 and n < per_slot:
                            dribble_proj()
                            owed -= 1.0
                            n += 1
                # drain any remaining proj work (shouldn't happen)
                while pending:
                    dribble_proj()

                # ---------------- output head ----------------
                pv = projps.tile([1, BL], F32, tag="pps")
                n_mm = 0
                for x in range(NC):
                    b0, bl = CH_OFF[x], CHAINS[x]
                    for kt in range(KT):
                        n_mm += 1
                        nc.tensor.matmul(
                            pv[0:1, b0 : b0 + bl],
                            vt_sb[:, kt : kt + 1],
                            h_cur[x][:, kt, :],
                            start=(n_mm == 1),
                            stop=(n_mm == NC * KT),
                            skip_group_check=True,
                        )
                # sigmoid(z+vb) == 0.5*tanh((z+vb)/2)+0.5; vb pre-halved, affine on host
                out_sb = miscp.tile([1, BL], F32, tag="out")
                nc.scalar.activation(out_sb[:], pv[:], AF.Tanh, bias=vb_sb[:], scale=0.5)
                nc.sync.dma_start(out=out_d[:], in_=out_sb[:])

    _strip_same_engine_waits(nc)
    nc.finalize()
    return nc


_ENGSEM = {"Activation": "Activation_", "PE": "PE_", "DVE": "DVE_", "Pool": "Pool_", "SP": "SP_"}


def _strip_same_engine_waits(nc):
    """Drop semaphore waits an instruction holds on its own engine's
    completion counter. Engines execute in order, so these waits are
    redundant for correctness; removing them lets the one remaining
    cross-engine wait attach to the instruction (instead of spilling to a
    standalone EventSemaphore that blocks the sequencer from pre-decoding),
    which cuts ~80ns of dispatch latency out of the per-step critical cycle."""
    if os.environ.get("V2_STRIP", "1") != "1":
        return
    for f in nc.m.functions:
        for b in f.blocks:
            for ins in b.instructions:
                si = ins.sync_info
                if si is None or not si.on_wait:
                    continue
                pref = _ENGSEM.get(str(ins.engine).split(".")[-1])
                if pref is None:
                    continue
                keep = [w for w in si.on_wait if not w.ant_name.startswith(pref)]
                if len(keep) != len(si.on_wait):
                    si.on_wait = keep


def kernel(x, emb, W_w, W_b, U_w, U_b, V_w, V_b):
    x = np.asarray(x)
    emb = np.asarray(emb, dtype=np.float32)
    W_w = np.asarray(W_w, dtype=np.float32)
    W_b = np.asarray(W_b, dtype=np.float32)
    U_w = np.asarray(U_w, dtype=np.float32)
    U_b = np.asarray(U_b, dtype=np.float32)
    V_w = np.asarray(V_w, dtype=np.float32)
    V_b = np.asarray(V_b, dtype=np.float32)

    if "nc" not in _cache:
        _cache["nc"] = _build()
    nc = _cache["nc"]

    bf = np.float16
    embt = np.ascontiguousarray(emb.astype(bf))
    # wt[p, et, h] = W_w.T[et*128+p, h]
    wt = np.ascontiguousarray(W_w.T.reshape(ET, P, H).transpose(1, 0, 2).astype(bf))
    # ut[p, kt, j] = U_w.T[kt*128+p, j]
    ut = np.ascontiguousarray(U_w.T.reshape(KT, P, H).transpose(1, 0, 2).astype(bf))
    bias = np.ascontiguousarray((W_b + U_b).reshape(HT, P).T.astype(np.float32))
    vt = np.ascontiguousarray(V_w[0].reshape(HT, P).T.astype(bf))
    vb = (V_b / 2.0).reshape(1, 1).astype(np.float32)
    ident = np.eye(P, dtype=np.float32).astype(bf)

    in_maps = []
    for c in range(NCORES):
        xl = np.ascontiguousarray(
            np.tile(x[c * BL : (c + 1) * BL, :].astype(np.int16), (P // BL, 1))
        )
        in_maps.append(
            {
                "embt": embt,
                "idx": xl,
                "wt": wt,
                "ut": ut,
                "bias": bias,
                "vt": vt,
                "vb": vb,
                "ident": ident,
            }
        )

    _cache["last_in_maps"] = in_maps
    res = run_bass_kernel_spmd(nc, in_maps, list(range(NCORES)))
    _cache["last_exec_time_ns"] = res.exec_time_ns
    _cache["last_results"] = res

    out = np.empty((B, 1), dtype=np.float32)
    for c in range(NCORES):
        out[c * BL : (c + 1) * BL, 0] = res.results[c]["out"][0, :]
    return 0.5 * out + 0.5


# revision 4
# speedup vs baseline: 1.0055x; 1.0004x over previous
"""Trainium2 Bass kernel v2: embedding -> Linear -> tanh-RNN -> Linear -> sigmoid.

Structure vs v1: the per-core batch of 16 rows is split into C independent
recurrence chains (default 6,5,5). Each chain-step is one PSUM accumulation
group (identity-seeded with p_t, then 64 U-matmuls) closed by a single ACT
tanh. Chains are emitted round-robin so each chain's matmul->tanh->matmul
latency (~680ns) is hidden under the other chains' matmuls; the input
projection (embedding gather + W matmul + bias via DVE) dribbles into the
leftover PE/DVE slack.

Hardcoded problem shapes:
  x   [128, 512] int   (token ids < 32000)
  emb [32000, 512] f32
  W_w [1024, 512], W_b [1024]
  U_w [1024, 1024], U_b [1024]
  V_w [1, 1024],  V_b [1]
"""

import os
import sys

import numpy as np

sys.path.insert(0, "/opt/trn_rl_repo")

import concourse.bass as bass  # noqa: E402
from concourse import bacc  # noqa: E402
import concourse.mybir as mybir  # noqa: E402
import concourse.tile as tile  # noqa: E402
from concourse.bass_utils import run_bass_kernel_spmd  # noqa: E402

B, S, E, H, VOCAB = 128, 512, 512, 1024, 32000
NCORES = 8
BL = B // NCORES  # 16 batch rows per core
NTOK = BL * S  # 8192 tokens per core, flat order i = s*BL + b
CHUNK = int(os.environ.get("V2_CHUNK", "128"))  # tokens per gather chunk
P = 128
ET, HT, KT = E // P, H // P, H // P  # 4, 8, 8
SPC = CHUNK // BL  # steps covered per chunk

F16 = mybir.dt.float16
F32 = mybir.dt.float32
I16 = mybir.dt.int16
AF = mybir.ActivationFunctionType

CHAINS = tuple(int(c) for c in os.environ.get("V2_CHAINS", "6,5,5").split(","))
assert sum(CHAINS) == BL
NC = len(CHAINS)
CH_OFF = [sum(CHAINS[:i]) for i in range(NC)]
# jt-split per chain: tanh'd in G groups so next step's kt-lo matmuls can
# start as soon as the first group's tanh lands
JSPLIT = tuple(int(g) for g in os.environ.get("V2_JSPLIT", "8").split(","))
assert sum(JSPLIT) == HT
JOFF = [sum(JSPLIT[:i]) for i in range(len(JSPLIT))]

STEPS = int(os.environ.get("V2_STEPS", S))
GATHER_AHEAD = int(os.environ.get("V2_AHEAD", "2"))
# how many proj matmuls to insert after each chain slot (<=0: all at step end)
COPY_ACT = os.environ.get("V2_COPY_ACT", "0") == "1"  # proj copies on ACT too

_cache = {}


def _build():
    nc = bacc.Bacc(None)
    emb_d = nc.declare_dram_parameter("embt", [VOCAB, E], F16, isOutput=False)
    idx_d = nc.declare_dram_parameter("idx", [P, S], I16, isOutput=False)
    wt_d = nc.declare_dram_parameter("wt", [P, ET, H], F16, isOutput=False)
    ut_d = nc.declare_dram_parameter("ut", [P, KT, H], F16, isOutput=False)
    bias_d = nc.declare_dram_parameter("bias", [P, HT], F32, isOutput=False)
    vt_d = nc.declare_dram_parameter("vt", [P, HT], F16, isOutput=False)
    vb_d = nc.declare_dram_parameter("vb", [1, 1], F32, isOutput=False)
    ident_d = nc.declare_dram_parameter("ident", [P, P], F16, isOutput=False)
    out_d = nc.declare_dram_parameter("out", [1, BL], F32, isOutput=True)

    with tile.TileContext(nc) as tc:
        with (
            tc.tile_pool(name="const", bufs=1) as constp,
            tc.tile_pool(name="pre", bufs=1) as prep,
            tc.tile_pool(name="xe", bufs=int(os.environ.get("V2_XEBUFS", "3"))) as xep,
            tc.tile_pool(name="h", bufs=int(os.environ.get("V2_HBUFS", "2"))) as hp,
            tc.tile_pool(name="misc", bufs=1) as miscp,
        ):
            # DMA order matters: everything the first projection chunk and
            # step-0 tanh needs goes first; the 2MB ut load is split per-kt
            # and issued last so it streams in under the prologue instead of
            # blocking it (DMA engine pool is serial in the cost model).
            idx_sb = constp.tile([P, S], I16, tag="idx")
            nc.sync.dma_start(out=idx_sb[:], in_=idx_d[:])
            wt_sb = constp.tile([P, ET, H], F16, tag="wt")
            nc.sync.dma_start(out=wt_sb[:], in_=wt_d[:])
            bias_sb = constp.tile([P, HT], F32, tag="bias")
            nc.sync.dma_start(out=bias_sb[:], in_=bias_d[:])
            ident_sb = constp.tile([P, P], F16, tag="ident")
            nc.sync.dma_start(out=ident_sb[:], in_=ident_d[:])
            vt_sb = constp.tile([P, HT], F16, tag="vt")
            nc.sync.dma_start(out=vt_sb[:], in_=vt_d[:])
            vb_sb = constp.tile([1, 1], F32, tag="vb")
            nc.sync.dma_start(out=vb_sb[:], in_=vb_d[:])
            ut_sb = constp.tile([P, KT, H], F16, tag="ut")
            nc.sync.dma_start(out=ut_sb[:, 0 : KT // 2, :], in_=ut_d[:, 0 : KT // 2, :])
            nc.sync.dma_start(out=ut_sb[:, KT // 2 :, :], in_=ut_d[:, KT // 2 :, :])

            # preT[p, ht, s*BL + b] = (xe @ W.T + W_b + U_b)[b, s, ht*128 + p]
            preT = prep.tile([P, HT, NTOK], F16, tag="preT")

            with (
                tc.tile_pool(
                    name="recps",
                    bufs=int(os.environ.get("V2_RECBUFS", "6")),
                    space=bass.MemorySpace.PSUM,
                ) as recps,
                tc.tile_pool(
                    name="projps", bufs=2, space=bass.MemorySpace.PSUM
                ) as projps,
            ):
                h_cur = [None] * NC

                def emit_gather(c):
                    xet = xep.tile([P, ET, CHUNK], F16, tag="xet")
                    nc.gpsimd.dma_gather(
                        out_ap=xet[:],
                        in_ap=emb_d[:],
                        idxs_ap=idx_sb[:, c * SPC : (c + 1) * SPC],
                        num_idxs=CHUNK,
                        num_idxs_reg=CHUNK,
                        elem_size=E,
                        transpose=True,
                    )
                    return xet

                def emit_proj_mm(xet, c, ht):
                    ps = projps.tile([P, CHUNK], F32, tag="pps")
                    for et in range(ET):
                        nc.tensor.matmul(
                            ps[:],
                            wt_sb[:, et, ht * P : (ht + 1) * P],
                            xet[:, et, 0:CHUNK],
                            start=(et == 0),
                            stop=(et == ET - 1),
                        )
                    return ps

                def emit_proj_one_mm(xet, ps, ht, et):
                    if et == 0:
                        ps = projps.tile([P, CHUNK], F32, tag="pps")
                    nc.tensor.matmul(
                        ps[:],
                        wt_sb[:, et, ht * P : (ht + 1) * P],
                        xet[:, et, 0:CHUNK],
                        start=(et == 0),
                        stop=(et == ET - 1),
                    )
                    return ps

                def emit_proj_copy(ps, c, ht):
                    toff = c * CHUNK
                    eng = nc.scalar if (COPY_ACT and ht % 2 == 0) else nc.vector
                    if eng is nc.vector:
                        nc.vector.tensor_tensor(
                            out=preT[:, ht, toff : toff + CHUNK],
                            in0=ps[:],
                            in1=bias_sb[:, ht : ht + 1].to_broadcast([P, CHUNK]),
                            op=mybir.AluOpType.add,
                        )
                    else:
                        nc.scalar.activation(
                            preT[:, ht, toff : toff + CHUNK],
                            ps[:],
                            AF.Identity,
                            bias=bias_sb[:, ht : ht + 1],
                        )

                def emit_chain_step(x, t):
                    b0, bl = CH_OFF[x], CHAINS[x]
                    if t == 0:
                        h_new = hp.tile([P, KT, bl], F16, tag=f"h{x}")
                        nc.scalar.activation(
                            h_new[:, :, :],
                            preT[:, :, b0 : b0 + bl],
                            AF.Tanh,
                        )
                        h_cur[x] = h_new
                        return
                    h_prev = h_cur[x]
                    h_new = hp.tile([P, KT, bl], F16, tag=f"h{x}")
                    toff = t * BL + b0
                    # one PSUM group + tanh per jt-group; matmuls kt-ascending so
                    # they start as soon as the previous step's first tanh lands
                    for gi, gsz in enumerate(JSPLIT):
                        j0 = JOFF[gi]
                        ps = recps.tile([P, max(JSPLIT), bl], F32, tag="recps")
                        nc.tensor.matmul(
                            ps[:, 0:gsz, :],
                            ident_sb[:],
                            preT[:, j0 : j0 + gsz, toff : toff + bl],
                            start=True,
                            stop=False,
                            skip_group_check=True,
                        )
                        n_mm = 0
                        for kt in range(KT):
                            for j in range(gsz):
                                n_mm += 1
                                nc.tensor.matmul(
                                    ps[:, j, :],
                                    ut_sb[:, kt, (j0 + j) * P : (j0 + j + 1) * P],
                                    h_prev[:, kt, :],
                                    start=False,
                                    stop=(n_mm == gsz * KT),
                                    skip_group_check=True,
                                )
                        nc.scalar.activation(
                            h_new[:, j0 : j0 + gsz, :], ps[:, 0:gsz, :], AF.Tanh
                        )
                    h_cur[x] = h_new

                # ---------------- pipelined emission ----------------
                # proj work for chunk c is emitted during steps of window c-1;
                # gather for chunk c+AHEAD-1 kicked at window start.
                NCHUNKS = NTOK // CHUNK

                # prologue: gather + project chunk 0 fully, gather chunk 1..AHEAD
                xet0 = emit_gather(0)
                for ht in range(HT):
                    ps = emit_proj_mm(xet0, 0, ht)
                    emit_proj_copy(ps, 0, ht)
                pending = []  # list of (xet, chunk, next_ht)
                for c in range(1, min(GATHER_AHEAD + 1, NCHUNKS)):
                    pending.append([emit_gather(c), c, 0])

                cur_ps = [None]

                def dribble_proj():
                    # one matmul per call; close+copy after ET of them
                    if not pending:
                        return
                    ent = pending[0]
                    xet, c, mmi = ent
                    ht, et = mmi // ET, mmi % ET
                    cur_ps[0] = emit_proj_one_mm(xet, cur_ps[0], ht, et)
                    if et == ET - 1:
                        emit_proj_copy(cur_ps[0], c, ht)
                    ent[2] += 1
                    if ent[2] == HT * ET:
                        pending.pop(0)

                # proj matmuls owed per step so gather chunks drain on schedule
                rate = (HT * ET) / SPC
                per_slot = max(1, int(rate / NC) + 1)
                owed = 0.0
                for t in range(STEPS):
                    if t % SPC == 0 and t > 0:
                        nxt = t // SPC + GATHER_AHEAD
                        if nxt < NCHUNKS:
                            pending.append([emit_gather(nxt), nxt, 0])
                    owed += rate
                    for x in range(NC):
                        emit_chain_step(x, t)
                        # insert proj matmuls between chain blocks so the
                        # tanh stream is spread across the period
                        n = 0
                        while t > 0 and owed >= 1.0 and n < per_slot:
                            dribble_proj()
                            owed -= 1.0
                            n += 1
                # drain any remaining proj work (shouldn't happen)
                while pending:
                    dribble_proj()

                # ---------------- output head ----------------
                pv = projps.tile([1, BL], F32, tag="pps")
                n_mm = 0
                for x in range(NC):
                    b0, bl = CH_OFF[x], CHAINS[x]
                    for kt in range(KT):
                        n_mm += 1
                        nc.tensor.matmul(
                            pv[0:1, b0 : b0 + bl],
                            vt_sb[:, kt : kt + 1],
                            h_cur[x][:, kt, :],
                            start=(n_mm == 1),
                            stop=(n_mm == NC * KT),
                            skip_group_check=True,
                        )
                # sigmoid(z+vb) == 0.5*tanh((z+vb)/2)+0.5; vb pre-halved, affine on host
                out_sb = miscp.tile([1, BL], F32, tag="out")
                nc.scalar.activation(out_sb[:], pv[:], AF.Tanh, bias=vb_sb[:], scale=0.5)
                nc.sync.dma_start(out=out_d[:], in_=out_sb[:])

    _strip_same_engine_waits(nc)
    nc.finalize()
    return nc


_ENGSEM = {"Activation": "Activation_", "PE": "PE_", "DVE": "DVE_", "Pool": "Pool_", "SP": "SP_"}


def _strip_same_engine_waits(nc):
    """Drop semaphore waits an instruction holds on its own engine's
    completion counter. Engines execute in order, so these waits are
    redundant for correctness; removing them lets the one remaining
    cross-engine wait attach to the instruction (instead of spilling to a
    standalone EventSemaphore that blocks the sequencer from pre-decoding),
    which cuts ~80ns of dispatch latency out of the per-step critical cycle."""
    if os.environ.get("V2_STRIP", "1") != "1":
        return
    for f in nc.m.functions:
        for b in f.blocks:
            for ins in b.instructions:
                si = ins.sync_info
                if si is None or not si.on_wait:
                    continue
                pref = _ENGSEM.get(str(ins.engine).split(".")[-1])
                if pref is None:
                    continue
                keep = [w for w in si.on_wait if not w.ant_name.startswith(pref)]
                if len(keep) != len(si.on_wait):
                    si.on_wait = keep


def kernel(x, emb, W_w, W_b, U_w, U_b, V_w, V_b):
    x = np.asarray(x)
    emb = np.asarray(emb, dtype=np.float32)
    W_w = np.asarray(W_w, dtype=np.float32)
    W_b = np.asarray(W_b, dtype=np.float32)
    U_w = np.asarray(U_w, dtype=np.float32)
    U_b = np.asarray(U_b, dtype=np.float32)
    V_w = np.asarray(V_w, dtype=np.float32)
    V_b = np.asarray(V_b, dtype=np.float32)

    if "nc" not in _cache:
        _cache["nc"] = _build()
    nc = _cache["nc"]

    bf = np.float16
    embt = np.ascontiguousarray(emb.astype(bf))
    # wt[p, et, h] = W_w.T[et*128+p, h]
    wt = np.ascontiguousarray(W_w.T.reshape(ET, P, H).transpose(1, 0, 2).astype(bf))
    # ut[p, kt, j] = U_w.T[kt*128+p, j]
    ut = np.ascontiguousarray(U_w.T.reshape(KT, P, H).transpose(1, 0, 2).astype(bf))
    bias = np.ascontiguousarray((W_b + U_b).reshape(HT, P).T.astype(np.float32))
    vt = np.ascontiguousarray(V_w[0].reshape(HT, P).T.astype(bf))
    vb = (V_b / 2.0).reshape(1, 1).astype(np.float32)
    ident = np.eye(P, dtype=np.float32).astype(bf)

    in_maps = []
    for c in range(NCORES):
        xl = np.ascontiguousarray(
            np.tile(x[c * BL : (c + 1) * BL, :].astype(np.int16), (P // BL, 1))
        )
        in_maps.append(
            {
                "embt": embt,
                "idx": xl,
                "wt": wt,
                "ut": ut,
                "bias": bias,
                "vt": vt,
                "vb": vb,
                "ident": ident,
            }
        )

    _cache["last_in_maps"] = in_maps
    res = run_bass_kernel_spmd(nc, in_maps, list(range(NCORES)))
    _cache["last_exec_time_ns"] = res.exec_time_ns
    _cache["last_results"] = res

    out = np.empty((B, 1), dtype=np.float32)
    for c in range(NCORES):
        out[c * BL : (c + 1) * BL, 0] = res.results[c]["out"][0, :]
    return 0.5 * out + 0.5
